# revision 32
# baseline (speedup 1.0000x reference)
"""Multi-head self-attention (B=4, S=2048, D=1024, H=16) on 8 TRN2 NeuronCores.

Sharding: data-parallel over batch x tensor-parallel over heads (Megatron
column-split of w_qkv, row-split of w_out). Core c computes batch c//2 with
heads (c%2)*8..(c%2)*8+8 and produces a partial [S, D] output; the host sums
the two partials per batch and adds the bias.

Per-core kernel (single Tile program, bf16 matmuls ~5e-3 rel err):
  - weights arrive as three single-DMA contiguous slabs (f32->bf16 converted
    in flight); x rows are DMA'd straight to bf16 and PE-transposed into a
    resident SBUF xT [d, S].
  - the 16 attention passes (4 head-pairs x 2 q-halves x 2 parities, q-half
    outer) run as ONE seamless k-tile stream: per k-tile the PE does
    QK (2 matmuls) + the PV pair two k-tiles behind, and the ACT engine
    exp-s the previous score tile; consecutive passes overlap (the first two
    k-tiles of pass p+1 carry the trailing PVs of pass p) so neither engine
    drains at a boundary.
  - only chunks 0-1 of the transpose/pair0-projection run before the stream;
    chunks 2-3 and all v-projections interleave into pass 0's k-tiles, so
    the PE chews attention while the x DMAs finish.
  - remaining work (later pairs' projections, out-projection rows) lives in
    a prerequisite-tagged FIFO sprinkled one item every few k-tiles to fill
    the PE's per-k-tile slack (ACT exp 1114ns vs 852ns of attention matmuls).
  - normalize: 1/denom (DVE) -> partition_broadcast (GPSIMD) -> multiply into
    the feat-major outT tile; pair-projection PSUM drains ride the ACT
    engine (which has slack in the PE-paced first q-half).
  - y = sum_pairs outT^T @ wout at K=128, sprinkled through the second
    q-half's passes with 3 rows reserved to cover the tail's normalize.
"""

import numpy as np

from concourse import bass_utils



from collections import deque
from contextlib import ExitStack

import concourse.bacc as bacc
import concourse.bass as bass
import concourse.mybir as mybir
import concourse.tile as tile
from concourse import masks

P = 128
HD = 64
HV = HD + 1
QCH = 512
F32 = mybir.dt.float32
F32R = mybir.dt.float32r
BF16 = mybir.dt.bfloat16
EXP = mybir.ActivationFunctionType.Exp


def build_attention(
    S: int,
    D: int,
    HN: int,
    DO: int,
    scale: float,
    dt_x=BF16,
    dt_qk=BF16,
    dt_e=BF16,
    dt_o=BF16,
) -> bacc.Bacc:
    F = HN * HD
    n_st = S // P
    n_dt = D // P
    n_ft = F // P
    n_ch = S // QCH
    n_kt = S // P
    n_no = DO // QCH
    QH = min(1024, S)
    n_qh = S // QH
    n_j = QH // QCH
    n_sti = QCH // P
    assert S % QCH == 0 and D % P == 0 and F % P == 0 and DO % QCH == 0
    assert mybir.dt.size(dt_x) == 2, "this build is bf16-only"

    nc = bacc.Bacc("TRN2", target_bir_lowering=False, debug=False)

    # inputs are pre-quantized to bf16 on the host: halves the HBM traffic
    # of the DMA-bound startup window and skips the on-chip casts
    x = nc.dram_tensor("x", [S, D], dt_x, kind="ExternalInput")
    wq = nc.dram_tensor("wq", [D, F], dt_x, kind="ExternalInput")
    wk = nc.dram_tensor("wk", [D, F], dt_x, kind="ExternalInput")
    wv = nc.dram_tensor("wv", [D, F], dt_x, kind="ExternalInput")
    wout = nc.dram_tensor("wout", [F, DO], dt_o, kind="ExternalInput")
    y = nc.dram_tensor("y", [S, DO], F32, kind="ExternalOutput")

    with tile.TileContext(nc) as tc, ExitStack() as top:  # noqa: PLR1702
        const_pool = top.enter_context(tc.tile_pool(name="const", bufs=1))
        ident = const_pool.tile([P, P], F32, tag="ident")
        masks.make_identity(nc, ident[:])
        ident_b = const_pool.tile([P, P], BF16, tag="identb")
        nc.vector.tensor_copy(ident_b[:], ident[:])
        ones_f32 = const_pool.tile([P, HD], F32, tag="ones_f32")
        nc.gpsimd.memset(ones_f32[:], 1.0)

        v_pool = top.enter_context(tc.tile_pool(name="vsb", bufs=1))
        v_sb = [
            v_pool.tile([P, HN * HV], dt_e, tag=f"v{st}", name=f"v_sb{st}")
            for st in range(n_st)
        ]
        for st in range(n_st):
            nc.vector.tensor_copy(
                v_sb[st][:].rearrange("p (h v) -> p h v", v=HV)[:, :, HD:].rearrange(
                    "p h one -> p (h one)"
                ),
                ones_f32[:, :HN],
            )

        outT_pool = top.enter_context(tc.tile_pool(name="outT", bufs=1))
        outP = [
            outT_pool.tile([P, S], dt_o, tag=f"o{ft}", name=f"outP{ft}")
            for ft in range(n_ft)
        ]

        # weight slabs: one [128, n_dt*F] tile per matrix, loaded by a single
        # DMA with 2KB source rows (f32 -> bf16 in flight)
        wqk_pool = top.enter_context(tc.tile_pool(name="wqk", bufs=1))
        wq_slab = wqk_pool.tile([P, n_dt * F], dt_x, tag="wq", name="wq_slab")
        wk_slab = wqk_pool.tile([P, n_dt * F], dt_x, tag="wk", name="wk_slab")
        wv_slab = wqk_pool.tile([P, n_dt * F], dt_x, tag="wv", name="wv_slab")
        wq_t = [
            [wq_slab[:, db * F + ft * P : db * F + (ft + 1) * P] for ft in range(n_ft)]
            for db in range(n_dt)
        ]
        wk_t = [
            [wk_slab[:, db * F + ft * P : db * F + (ft + 1) * P] for ft in range(n_ft)]
            for db in range(n_dt)
        ]
        wv_t = [wv_slab[:, db * F : (db + 1) * F] for db in range(n_dt)]
        for slab, src in ((wq_slab, wq), (wk_slab, wk), (wv_slab, wv)):
            nc.gpsimd.dma_start(
                slab[:].rearrange("p (db c) -> p db c", c=F),
                src.rearrange("(db p) c -> p db c", p=P),
            )

        # all four pairs stay resident (q-half-outer pass order reuses them)
        pair_pool = top.enter_context(tc.tile_pool(name="pair", bufs=n_ft))
        pair_tiles = {}

        def get_pair(ft):
            if ft not in pair_tiles:
                pair_tiles[ft] = (
                    pair_pool.tile([P, S], dt_qk, tag="qp", name=f"qTp{ft}"),
                    pair_pool.tile([P, S], dt_qk, tag="kp", name=f"kTp{ft}"),
                )
            return pair_tiles[ft]

        # x^T kept resident in SBUF for the whole run (no DRAM roundtrip)
        xT_pool = top.enter_context(tc.tile_pool(name="xTsb", bufs=1))
        xT_sb = [
            xT_pool.tile([P, S], dt_x, tag=f"xT{db}", name=f"xTsb{db}")
            for db in range(n_dt)
        ]

        ps_sc = top.enter_context(
            tc.tile_pool(name="ps_sc", bufs=3, space=bass.MemorySpace.PSUM)
        )
        ps_pv = top.enter_context(
            tc.tile_pool(name="ps_pv", bufs=1, space=bass.MemorySpace.PSUM)
        )
        e_pool = top.enter_context(tc.tile_pool(name="epool", bufs=4))
        stg_pool = top.enter_context(tc.tile_pool(name="stgpool", bufs=3))
        rc_pool = top.enter_context(tc.tile_pool(name="rcpool", bufs=2))
        bcs_pool = top.enter_context(tc.tile_pool(name="bcspool", bufs=2))
        # x row staging, released once the last chunk is transposed (created
        # last for pool stack order)
        xup_stack = ExitStack()
        xst_pool = xup_stack.enter_context(tc.tile_pool(name="xst", bufs=2 * n_sti))

        # ---------------- building blocks ----------------
        def upfront_chunk(ch, qTp0, kTp0):
            xrows = []
            for sti in range(n_sti):
                st = ch * n_sti + sti
                xrow = xst_pool.tile([P, D], dt_x, tag="xrow", name=f"xrow{st}")
                nc.sync.dma_start(xrow[:], x[st * P : (st + 1) * P, :])
                xrows.append(xrow)
            xT = [xT_sb[db][:, ch * QCH : (ch + 1) * QCH] for db in range(n_dt)]
            for db in range(n_dt):
                tp = ps_sc.tile([P, QCH], dt_x, tag="sc", name=f"tr{ch}_{db}")
                for sti in range(n_sti):
                    nc.tensor.transpose(
                        tp[:, sti * P : (sti + 1) * P],
                        xrows[sti][:, db * P : (db + 1) * P],
                        ident_b[:],
                    )
                # PSUM->SBUF drains alternate between ACT and DVE so the
                # xT chunk is ready sooner (GPSIMD cannot access PSUM)
                if db % 2 == 0:
                    nc.scalar.copy(xT[db], tp[:])
                else:
                    nc.vector.tensor_copy(xT[db], tp[:])
            for w_t, dstp in ((wq_t, qTp0), (wk_t, kTp0)):
                pp = ps_sc.tile([P, QCH], F32, tag="sc", name=f"pj0_{ch}")
                for db in range(n_dt):
                    nc.tensor.matmul(
                        pp[:],
                        w_t[db][0],
                        xT[db],
                        start=(db == 0),
                        stop=(db == n_dt - 1),
                    )
                nc.vector.tensor_copy(dstp[:, ch * QCH : (ch + 1) * QCH], pp[:])

        def v_chunk(st):
            pv_ps = ps_sc.tile([P, F], F32, tag="sc", name=f"pvp{st}")
            for db in range(n_dt):
                nc.tensor.matmul(
                    pv_ps[:],
                    xT_sb[db][:, st * P : (st + 1) * P],
                    wv_t[db],
                    start=(db == 0),
                    stop=(db == n_dt - 1),
                )
            nc.vector.tensor_copy(
                v_sb[st][:].rearrange("p (h v) -> p h v", v=HV)[:, :, :HD],
                pv_ps[:].rearrange("p (h d) -> p h d", d=HD),
            )

        def proj_items(ftn):
            """Matmul closures projecting pair ftn's qT/kT from resident xT."""
            qTp, kTp = get_pair(ftn)

            def mm_item(ch, w_t, dstp, which):
                def run():
                    pp = ps_sc.tile([P, QCH], F32, tag="sc", name=f"pj{which}{ftn}_{ch}")
                    for db in range(n_dt):
                        nc.tensor.matmul(
                            pp[:],
                            w_t[db][ftn],
                            xT_sb[db][:, ch * QCH : (ch + 1) * QCH],
                            start=(db == 0),
                            stop=(db == n_dt - 1),
                        )
                    # PSUM->SBUF drain on ACT: during the proj-heavy first
                    # q-half the passes are PE-paced, so ACT has slack, while
                    # the DVE queue would delay this behind 3.3us reciprocals
                    nc.scalar.copy(dstp[:, ch * QCH : (ch + 1) * QCH], pp[:])

                return run

            items = []
            for ch in range(n_ch):
                items.append(mm_item(ch, wk_t, kTp, "k"))
                items.append(mm_item(ch, wq_t, qTp, "q"))
            return items

        class AttnQH:
            """Emitter for one (pair, q-half, head-parity) attention pass,
            driven one k-tile at a time by the global stream. PV trails QK by
            LAG k-tiles so the ACT exp chain never stalls the PE."""

            LAG = 2

            def __init__(self, ft, qh, parity):
                self.ft, self.qh, self.parity = ft, qh, parity
                self.qTp, self.kTp = get_pair(ft)
                self.h = 2 * ft + parity
                self.q_base = qh * QH
                self.pv = ps_pv.tile(
                    [HV, QH], F32, tag="pv", name=f"pv{ft}_{qh}_{parity}"
                )
                self.prevs = deque()

            def emit_qk(self, kt):
                sub = self.parity * HD
                sc = ps_sc.tile(
                    [P, QH], F32, tag="sc",
                    name=f"sc{self.ft}{self.parity}{self.qh}{kt}",
                )
                for j in range(n_j):
                    q0 = self.q_base + j * QCH
                    nc.tensor.matmul(
                        sc[:, j * QCH : (j + 1) * QCH],
                        self.kTp[sub : sub + HD, kt * P : (kt + 1) * P],
                        self.qTp[sub : sub + HD, q0 : q0 + QCH],
                        start=True,
                        stop=True,
                    )
                et = e_pool.tile(
                    [P, QH], dt_e, tag="et",
                    name=f"e{self.ft}{self.parity}{self.qh}{kt}",
                )
                nc.scalar.activation(et[:], sc[:], EXP, scale=scale)
                self.prevs.append((kt, et))

            def emit_pv_one(self):
                kt, et = self.prevs.popleft()
                vt = v_sb[kt][:].rearrange("p (hh v) -> p hh v", v=HV)[:, self.h, :]
                for j in range(n_j):
                    nc.tensor.matmul(
                        self.pv[:, j * QCH : (j + 1) * QCH],
                        vt,
                        et[:, j * QCH : (j + 1) * QCH],
                        start=(kt == 0),
                        stop=(kt == n_kt - 1),
                    )
                return not self.prevs

            def finish_stage1(self):
                """Copy pv to SBUF staging (frees the PSUM accumulator)."""
                ft, qh, parity = self.ft, self.qh, self.parity
                self.stg = stg_pool.tile(
                    [HV, QH], F32, tag="stg", name=f"st{ft}{parity}{qh}"
                )
                nc.vector.tensor_copy(self.stg[:], self.pv[:])

            def normalize_items(self, gran=QCH):
                """Per-chunk normalize closures (reciprocal + broadcast +
                multiply); DVE/GPSIMD work, no PE instructions. `gran` sets
                the chunk width (finer for the tail, where the reciprocal's
                serial latency gates the final out-projection rows)."""
                ft, qh, parity, q_base = self.ft, self.qh, self.parity, self.q_base
                stg = self.stg

                def norm_item(qc):
                    def run():
                        rc = rc_pool.tile(
                            [1, gran], F32, tag="rc", name=f"rc{ft}{parity}{qh}{qc}"
                        )
                        nc.vector.reciprocal(
                            rc[:], stg[HD : HD + 1, qc * gran : (qc + 1) * gran]
                        )
                        bcs = bcs_pool.tile(
                            [HD, gran], F32, tag="bcs", name=f"bc{ft}{parity}{qh}{qc}"
                        )
                        nc.gpsimd.partition_broadcast(bcs[:], rc[:])
                        with nc.allow_low_precision(reason="attn out cast"):
                            nc.gpsimd.tensor_mul(
                                outP[ft][
                                    parity * HD : (parity + 1) * HD,
                                    q_base + qc * gran : q_base + (qc + 1) * gran,
                                ],
                                stg[:HD, qc * gran : (qc + 1) * gran],
                                bcs[:],
                            )

                    return run

                return [norm_item(qc) for qc in range(QH // gran)]

        wo_t = []
        ys_pool_ref = []

        def y_items(qt_range, tail=False):
            def y_item(qt):
                def run():
                    for no in range(n_no):
                        yp = ps_sc.tile([P, QCH], F32, tag="sc", name=f"yp{qt}_{no}")
                        for ft in range(n_ft):
                            nc.tensor.matmul(
                                yp[:],
                                outP[ft][:, qt * P : (qt + 1) * P],
                                wo_t[ft][:, no * QCH : (no + 1) * QCH],
                                start=(ft == 0),
                                stop=(ft == n_ft - 1),
                            )
                        ys = ys_pool_ref[0].tile(
                            [P, QCH], F32, tag="ys", name=f"ys{qt}_{no}"
                        )
                        if tail:
                            # ACT is done with exps by the tail; keep the
                            # drain off the DVE queue (busy with reciprocals)
                            nc.scalar.copy(ys[:], yp[:])
                        else:
                            nc.vector.tensor_copy(ys[:], yp[:])
                        nc.sync.dma_start(
                            y[qt * P : (qt + 1) * P, no * QCH : (no + 1) * QCH], ys[:]
                        )

                return run

            return [y_item(qt) for qt in qt_range]

        # ---------------- emission ----------------
        qTp0, kTp0 = get_pair(0)
        upfront_chunk(0, qTp0, kTp0)
        upfront_chunk(1, qTp0, kTp0)

        # global sprinkle FIFO of (must_emit_before_pass_idx, closure)
        work = deque()
        for ftn in range(1, n_ft):
            for it in proj_items(ftn):
                work.append((2 * ftn, it))

        rows_per_qh = n_st // n_qh
        pass_specs = [
            (qh, ft, parity)
            for qh in range(n_qh)
            for ft in range(n_ft)
            for parity in (0, 1)
        ]
        n_passes = len(pass_specs)
        NEVER = n_passes + 1

        prev = None
        pending_norms = []
        enqueue_next = []  # items appended to `work` at the next pass boundary
        for pidx, (qh, ft, parity) in enumerate(pass_specs):
            for it in enqueue_next:
                work.append((NEVER, it))
            enqueue_next = []
            # force-emit overdue prerequisites (pair projections)
            while work and work[0][0] <= pidx:
                work.popleft()[1]()
            a = AttnQH(ft, qh, parity)
            if pidx == 0:
                stride = 5
            elif qh == 0:
                stride = 3
            else:
                stride = 14
            for kt in range(n_kt):
                if pidx == 0:
                    # finish the upfront while attention runs: remaining
                    # transpose chunks + the v-projections, one per k-tile
                    if n_ch > 2 and kt == 8:
                        upfront_chunk(2, qTp0, kTp0)
                    if n_ch > 3 and kt == 12:
                        upfront_chunk(3, qTp0, kTp0)
                    v_chunk(kt)
                if pending_norms and kt > 0 and kt % 6 == 0:
                    pending_norms.pop(0)()
                if work and kt % stride == stride - 1:
                    work.popleft()[1]()
                a.emit_qk(kt)
                if prev is not None:
                    if prev.emit_pv_one():
                        prev.finish_stage1()
                        pending_norms += prev.normalize_items()
                        if (prev.qh, prev.ft, prev.parity) == (0, n_ft - 1, 1):
                            # first q-half fully normalized soon: release its
                            # out-projection rows into the stream (keep 3 in
                            # reserve to cover the tail's normalize latency)
                            enqueue_next += y_items(range(rows_per_qh - 3))
                        prev = None
                elif len(a.prevs) > AttnQH.LAG:
                    a.emit_pv_one()
            if pidx == 0:
                # x fully transposed: release the row staging, then stage the
                # out-projection weights (first needed half-way through)
                xup_stack.close()
                wo_pool = top.enter_context(tc.tile_pool(name="wo", bufs=1))
                ys_pool_ref.append(
                    top.enter_context(tc.tile_pool(name="ys", bufs=3))
                )
                wo_t.extend(
                    wo_pool.tile([P, DO], dt_o, tag=f"wo{ft2}", name=f"wo{ft2}")
                    for ft2 in range(n_ft)
                )
                for ft2 in range(n_ft):
                    nc.gpsimd.dma_start(
                        wo_t[ft2][:], wout[ft2 * P : (ft2 + 1) * P, :]
                    )
            prev = a

        # tail: flush the last pass, run leftovers, last normalizes, last rows
        while prev.prevs:
            if work:
                work.popleft()[1]()
            prev.emit_pv_one()
        prev.finish_stage1()
        tail_norms = pending_norms + prev.normalize_items(gran=QCH // 2)
        leftovers = [it for _, it in work] + y_items(
            range(rows_per_qh - 3, rows_per_qh), tail=True
        )
        # alternate fine norms with reserved out-projection rows so the PE
        # chews while the DVE reciprocal chain completes
        while tail_norms or leftovers:
            if tail_norms:
                tail_norms.pop(0)()
            if leftovers:
                leftovers.pop(0)()
        for it in y_items(range(rows_per_qh, n_st), tail=True):
            it()

    nc.compile()
    return nc


# problem sizes (hardcoded per contract)
B, S, D, H = 4, 2048, 1024, 16
DO = D
HN = H // 2  # heads per core
SCALE = (D // H) ** -0.5
N_CORES = 8

_NC_CACHE = None


def _get_nc():
    global _NC_CACHE
    if _NC_CACHE is None:
        _NC_CACHE = build_attention(S, D, HN, DO, SCALE)
    return _NC_CACHE


def make_in_maps(x, w_qkv, w_out):
    """Shard full inputs into the 8 per-core input maps (bf16 on host)."""
    import ml_dtypes

    bf16 = ml_dtypes.bfloat16
    xb = x.astype(bf16)
    wb = w_qkv.astype(bf16)
    wob = w_out.astype(bf16)
    in_maps = []
    for c in range(N_CORES):
        b = c // 2
        cs = (c % 2) * HN * HD
        ce = cs + HN * HD
        in_maps.append(
            {
                "x": np.ascontiguousarray(xb[b]),
                "wq": np.ascontiguousarray(wb[:, cs:ce]),
                "wk": np.ascontiguousarray(wb[:, D + cs : D + ce]),
                "wv": np.ascontiguousarray(wb[:, 2 * D + cs : 2 * D + ce]),
                "wout": np.ascontiguousarray(wob[cs:ce, :]),
            }
        )
    return in_maps


def combine_outputs(results, b_out):
    """Sum the two per-batch partials and add the bias."""
    y = np.empty((B, S, DO), dtype=np.float32)
    for b in range(B):
        y[b] = results[2 * b]["y"] + results[2 * b + 1]["y"] + b_out[None, :]
    return y


def kernel(x, w_qkv, w_out, b_out):
    x = np.asarray(x, dtype=np.float32)
    w_qkv = np.asarray(w_qkv, dtype=np.float32)
    w_out = np.asarray(w_out, dtype=np.float32)
    b_out = np.asarray(b_out, dtype=np.float32)
    nc = _get_nc()
    in_maps = make_in_maps(x, w_qkv, w_out)
    res = bass_utils.run_bass_kernel_spmd(nc, in_maps, core_ids=list(range(N_CORES)))
    return combine_outputs(res.results, b_out)


# revision 33
# speedup vs baseline: 1.4168x; 1.4168x over previous
"""Multi-head self-attention (B=4, S=2048, D=1024, H=16) on 8 TRN2 NeuronCores.

Sharding: data-parallel over batch x tensor-parallel over heads (Megatron
column-split of w_qkv, row-split of w_out). Core c computes batch c//2 with
heads (c%2)*8..(c%2)*8+8 and produces a partial [S, D] output; the host sums
the two partials per batch and adds the bias.

Per-core kernel (single Tile program, bf16 matmuls ~5e-3 rel err):
  - weights arrive as three single-DMA contiguous slabs (f32->bf16 converted
    in flight); x rows are DMA'd straight to bf16 and PE-transposed into a
    resident SBUF xT [d, S].
  - the 16 attention passes (4 head-pairs x 2 q-halves x 2 parities, q-half
    outer) run as ONE seamless k-tile stream: per k-tile the PE does
    QK (2 matmuls) + the PV pair two k-tiles behind, and the ACT engine
    exp-s the previous score tile; consecutive passes overlap (the first two
    k-tiles of pass p+1 carry the trailing PVs of pass p) so neither engine
    drains at a boundary.
  - only chunks 0-1 of the transpose/pair0-projection run before the stream;
    chunks 2-3 and all v-projections interleave into pass 0's k-tiles, so
    the PE chews attention while the x DMAs finish.
  - remaining work (later pairs' projections, out-projection rows) lives in
    a prerequisite-tagged FIFO sprinkled one item every few k-tiles to fill
    the PE's per-k-tile slack (ACT exp 1114ns vs 852ns of attention matmuls).
  - normalize: 1/denom (DVE) -> partition_broadcast (GPSIMD) -> multiply into
    the feat-major outT tile; pair-projection PSUM drains ride the ACT
    engine (which has slack in the PE-paced first q-half).
  - y = sum_pairs outT^T @ wout at K=128, sprinkled through the second
    q-half's passes with 3 rows reserved to cover the tail's normalize.
"""

import numpy as np

from concourse import bass_utils



from collections import deque
from contextlib import ExitStack

import concourse.bacc as bacc
import concourse.bass as bass
import concourse.mybir as mybir
import concourse.tile as tile
from concourse import masks

P = 128
HD = 64
HV = HD + 1
QCH = 512
F32 = mybir.dt.float32
F32R = mybir.dt.float32r
BF16 = mybir.dt.bfloat16
EXP = mybir.ActivationFunctionType.Exp


def build_attention(
    S: int,
    D: int,
    HN: int,
    DO: int,
    scale: float,
    dt_x=BF16,
    dt_qk=BF16,
    dt_e=BF16,
    dt_o=BF16,
) -> bacc.Bacc:
    F = HN * HD
    n_st = S // P
    n_dt = D // P
    n_ft = F // P
    n_ch = S // QCH
    n_kt = S // P
    n_no = DO // QCH
    QH = min(1024, S)
    n_qh = S // QH
    n_j = QH // QCH
    n_sti = QCH // P
    assert S % QCH == 0 and D % P == 0 and F % P == 0 and DO % QCH == 0
    assert mybir.dt.size(dt_x) == 2, "this build is bf16-only"

    nc = bacc.Bacc("TRN2", target_bir_lowering=False, debug=False)

    # inputs are pre-quantized to bf16 on the host: halves the HBM traffic
    # of the DMA-bound startup window and skips the on-chip casts
    x = nc.dram_tensor("x", [S, D], dt_x, kind="ExternalInput")
    wq = nc.dram_tensor("wq", [D, F], dt_x, kind="ExternalInput")
    wk = nc.dram_tensor("wk", [D, F], dt_x, kind="ExternalInput")
    wv = nc.dram_tensor("wv", [D, F], dt_x, kind="ExternalInput")
    wout = nc.dram_tensor("wout", [F, DO], dt_o, kind="ExternalInput")
    y = nc.dram_tensor("y", [S, DO], F32, kind="ExternalOutput")

    with tile.TileContext(nc) as tc, ExitStack() as top:  # noqa: PLR1702
        const_pool = top.enter_context(tc.tile_pool(name="const", bufs=1))
        ident = const_pool.tile([P, P], F32, tag="ident")
        masks.make_identity(nc, ident[:])
        ident_b = const_pool.tile([P, P], BF16, tag="identb")
        nc.vector.tensor_copy(ident_b[:], ident[:])
        ones_f32 = const_pool.tile([P, HD], F32, tag="ones_f32")
        nc.gpsimd.memset(ones_f32[:], 1.0)

        v_pool = top.enter_context(tc.tile_pool(name="vsb", bufs=1))
        v_sb = [
            v_pool.tile([P, HN * HV], dt_e, tag=f"v{st}", name=f"v_sb{st}")
            for st in range(n_st)
        ]
        for st in range(n_st):
            nc.vector.tensor_copy(
                v_sb[st][:].rearrange("p (h v) -> p h v", v=HV)[:, :, HD:].rearrange(
                    "p h one -> p (h one)"
                ),
                ones_f32[:, :HN],
            )

        outT_pool = top.enter_context(tc.tile_pool(name="outT", bufs=1))
        outP = [
            outT_pool.tile([P, S], dt_o, tag=f"o{ft}", name=f"outP{ft}")
            for ft in range(n_ft)
        ]

        # weight slabs: one [128, n_dt*F] tile per matrix, loaded by a single
        # DMA with 2KB source rows (f32 -> bf16 in flight)
        wqk_pool = top.enter_context(tc.tile_pool(name="wqk", bufs=1))
        wq_slab = wqk_pool.tile([P, n_dt * F], dt_x, tag="wq", name="wq_slab")
        wk_slab = wqk_pool.tile([P, n_dt * F], dt_x, tag="wk", name="wk_slab")
        wv_slab = wqk_pool.tile([P, n_dt * F], dt_x, tag="wv", name="wv_slab")
        wq_t = [
            [wq_slab[:, db * F + ft * P : db * F + (ft + 1) * P] for ft in range(n_ft)]
            for db in range(n_dt)
        ]
        wk_t = [
            [wk_slab[:, db * F + ft * P : db * F + (ft + 1) * P] for ft in range(n_ft)]
            for db in range(n_dt)
        ]
        wv_t = [wv_slab[:, db * F : (db + 1) * F] for db in range(n_dt)]
        for slab, src in ((wq_slab, wq), (wk_slab, wk), (wv_slab, wv)):
            nc.gpsimd.dma_start(
                slab[:].rearrange("p (db c) -> p db c", c=F),
                src.rearrange("(db p) c -> p db c", p=P),
            )

        # all four pairs stay resident (q-half-outer pass order reuses them)
        pair_pool = top.enter_context(tc.tile_pool(name="pair", bufs=n_ft))
        pair_tiles = {}

        def get_pair(ft):
            if ft not in pair_tiles:
                pair_tiles[ft] = (
                    pair_pool.tile([P, S], dt_qk, tag="qp", name=f"qTp{ft}"),
                    pair_pool.tile([P, S], dt_qk, tag="kp", name=f"kTp{ft}"),
                )
            return pair_tiles[ft]

        # x^T kept resident in SBUF for the whole run (no DRAM roundtrip)
        xT_pool = top.enter_context(tc.tile_pool(name="xTsb", bufs=1))
        xT_sb = [
            xT_pool.tile([P, S], dt_x, tag=f"xT{db}", name=f"xTsb{db}")
            for db in range(n_dt)
        ]

        ps_sc = top.enter_context(
            tc.tile_pool(name="ps_sc", bufs=3, space=bass.MemorySpace.PSUM)
        )
        ps_pv = top.enter_context(
            tc.tile_pool(name="ps_pv", bufs=1, space=bass.MemorySpace.PSUM)
        )
        e_pool = top.enter_context(tc.tile_pool(name="epool", bufs=4))
        stg_pool = top.enter_context(tc.tile_pool(name="stgpool", bufs=3))
        rc_pool = top.enter_context(tc.tile_pool(name="rcpool", bufs=2))
        bcs_pool = top.enter_context(tc.tile_pool(name="bcspool", bufs=2))
        # x row staging, released once the last chunk is transposed (created
        # last for pool stack order)
        xup_stack = ExitStack()
        xst_pool = xup_stack.enter_context(tc.tile_pool(name="xst", bufs=2 * n_sti))

        # ---------------- building blocks ----------------
        def upfront_chunk(ch, qTp0, kTp0):
            xrows = []
            for sti in range(n_sti):
                st = ch * n_sti + sti
                xrow = xst_pool.tile([P, D], dt_x, tag="xrow", name=f"xrow{st}")
                nc.sync.dma_start(xrow[:], x[st * P : (st + 1) * P, :])
                xrows.append(xrow)
            xT = [xT_sb[db][:, ch * QCH : (ch + 1) * QCH] for db in range(n_dt)]
            for db in range(n_dt):
                tp = ps_sc.tile([P, QCH], dt_x, tag="sc", name=f"tr{ch}_{db}")
                for sti in range(n_sti):
                    nc.tensor.transpose(
                        tp[:, sti * P : (sti + 1) * P],
                        xrows[sti][:, db * P : (db + 1) * P],
                        ident_b[:],
                    )
                # PSUM->SBUF drains alternate between ACT and DVE so the
                # xT chunk is ready sooner (GPSIMD cannot access PSUM)
                if db % 2 == 0:
                    nc.scalar.copy(xT[db], tp[:])
                else:
                    nc.vector.tensor_copy(xT[db], tp[:])
            for w_t, dstp in ((wq_t, qTp0), (wk_t, kTp0)):
                pp = ps_sc.tile([P, QCH], F32, tag="sc", name=f"pj0_{ch}")
                for db in range(n_dt):
                    nc.tensor.matmul(
                        pp[:],
                        w_t[db][0],
                        xT[db],
                        start=(db == 0),
                        stop=(db == n_dt - 1),
                    )
                nc.vector.tensor_copy(dstp[:, ch * QCH : (ch + 1) * QCH], pp[:])

        def v_chunk(st):
            pv_ps = ps_sc.tile([P, F], F32, tag="sc", name=f"pvp{st}")
            for db in range(n_dt):
                nc.tensor.matmul(
                    pv_ps[:],
                    xT_sb[db][:, st * P : (st + 1) * P],
                    wv_t[db],
                    start=(db == 0),
                    stop=(db == n_dt - 1),
                )
            nc.vector.tensor_copy(
                v_sb[st][:].rearrange("p (h v) -> p h v", v=HV)[:, :, :HD],
                pv_ps[:].rearrange("p (h d) -> p h d", d=HD),
            )

        def proj_items(ftn):
            """Matmul closures projecting pair ftn's qT/kT from resident xT."""
            qTp, kTp = get_pair(ftn)

            def mm_item(ch, w_t, dstp, which):
                def run():
                    pp = ps_sc.tile([P, QCH], F32, tag="sc", name=f"pj{which}{ftn}_{ch}")
                    for db in range(n_dt):
                        nc.tensor.matmul(
                            pp[:],
                            w_t[db][ftn],
                            xT_sb[db][:, ch * QCH : (ch + 1) * QCH],
                            start=(db == 0),
                            stop=(db == n_dt - 1),
                        )
                    # PSUM->SBUF drain on ACT: during the proj-heavy first
                    # q-half the passes are PE-paced, so ACT has slack, while
                    # the DVE queue would delay this behind 3.3us reciprocals
                    nc.scalar.copy(dstp[:, ch * QCH : (ch + 1) * QCH], pp[:])

                return run

            items = []
            for ch in range(n_ch):
                items.append(mm_item(ch, wk_t, kTp, "k"))
                items.append(mm_item(ch, wq_t, qTp, "q"))
            return items

        class AttnQH:
            """Emitter for one (pair, q-half, head-parity) attention pass,
            driven one k-tile at a time by the global stream. PV trails QK by
            LAG k-tiles so the ACT exp chain never stalls the PE."""

            LAG = 2

            def __init__(self, ft, qh, parity):
                self.ft, self.qh, self.parity = ft, qh, parity
                self.qTp, self.kTp = get_pair(ft)
                self.h = 2 * ft + parity
                self.q_base = qh * QH
                self.pv = ps_pv.tile(
                    [HV, QH], F32, tag="pv", name=f"pv{ft}_{qh}_{parity}"
                )
                self.prevs = deque()

            def emit_qk(self, kt):
                sub = self.parity * HD
                sc = ps_sc.tile(
                    [P, QH], F32, tag="sc",
                    name=f"sc{self.ft}{self.parity}{self.qh}{kt}",
                )
                for j in range(n_j):
                    q0 = self.q_base + j * QCH
                    nc.tensor.matmul(
                        sc[:, j * QCH : (j + 1) * QCH],
                        self.kTp[sub : sub + HD, kt * P : (kt + 1) * P],
                        self.qTp[sub : sub + HD, q0 : q0 + QCH],
                        start=True,
                        stop=True,
                    )
                et = e_pool.tile(
                    [P, QH], dt_e, tag="et",
                    name=f"e{self.ft}{self.parity}{self.qh}{kt}",
                )
                nc.scalar.activation(et[:], sc[:], EXP, scale=scale)
                self.prevs.append((kt, et))

            def emit_pv_one(self):
                kt, et = self.prevs.popleft()
                vt = v_sb[kt][:].rearrange("p (hh v) -> p hh v", v=HV)[:, self.h, :]
                for j in range(n_j):
                    nc.tensor.matmul(
                        self.pv[:, j * QCH : (j + 1) * QCH],
                        vt,
                        et[:, j * QCH : (j + 1) * QCH],
                        start=(kt == 0),
                        stop=(kt == n_kt - 1),
                    )
                return not self.prevs

            def finish_stage1(self):
                """Copy pv to SBUF staging (frees the PSUM accumulator)."""
                ft, qh, parity = self.ft, self.qh, self.parity
                self.stg = stg_pool.tile(
                    [HV, QH], F32, tag="stg", name=f"st{ft}{parity}{qh}"
                )
                nc.vector.tensor_copy(self.stg[:], self.pv[:])

            def normalize_items(self, gran=QCH):
                """Per-chunk normalize closures (reciprocal + broadcast +
                multiply); DVE/GPSIMD work, no PE instructions. `gran` sets
                the chunk width (finer for the tail, where the reciprocal's
                serial latency gates the final out-projection rows)."""
                ft, qh, parity, q_base = self.ft, self.qh, self.parity, self.q_base
                stg = self.stg

                def norm_item(qc):
                    def run():
                        rc = rc_pool.tile(
                            [1, gran], F32, tag="rc", name=f"rc{ft}{parity}{qh}{qc}"
                        )
                        nc.vector.reciprocal(
                            rc[:], stg[HD : HD + 1, qc * gran : (qc + 1) * gran]
                        )
                        bcs = bcs_pool.tile(
                            [HD, gran], F32, tag="bcs", name=f"bc{ft}{parity}{qh}{qc}"
                        )
                        nc.gpsimd.partition_broadcast(bcs[:], rc[:])
                        with nc.allow_low_precision(reason="attn out cast"):
                            nc.vector.tensor_mul(
                                outP[ft][
                                    parity * HD : (parity + 1) * HD,
                                    q_base + qc * gran : q_base + (qc + 1) * gran,
                                ],
                                stg[:HD, qc * gran : (qc + 1) * gran],
                                bcs[:],
                            )

                    return run

                return [norm_item(qc) for qc in range(QH // gran)]

        wo_t = []
        ys_pool_ref = []

        def y_items(qt_range, tail=False):
            def y_item(qt):
                def run():
                    for no in range(n_no):
                        yp = ps_sc.tile([P, QCH], F32, tag="sc", name=f"yp{qt}_{no}")
                        for ft in range(n_ft):
                            nc.tensor.matmul(
                                yp[:],
                                outP[ft][:, qt * P : (qt + 1) * P],
                                wo_t[ft][:, no * QCH : (no + 1) * QCH],
                                start=(ft == 0),
                                stop=(ft == n_ft - 1),
                            )
                        ys = ys_pool_ref[0].tile(
                            [P, QCH], F32, tag="ys", name=f"ys{qt}_{no}"
                        )
                        if tail:
                            # ACT is done with exps by the tail; keep the
                            # drain off the DVE queue (busy with reciprocals)
                            nc.scalar.copy(ys[:], yp[:])
                        else:
                            nc.vector.tensor_copy(ys[:], yp[:])
                        nc.sync.dma_start(
                            y[qt * P : (qt + 1) * P, no * QCH : (no + 1) * QCH], ys[:]
                        )

                return run

            return [y_item(qt) for qt in qt_range]

        # ---------------- emission ----------------
        qTp0, kTp0 = get_pair(0)
        upfront_chunk(0, qTp0, kTp0)
        upfront_chunk(1, qTp0, kTp0)

        # global sprinkle FIFO of (must_emit_before_pass_idx, closure)
        work = deque()
        for ftn in range(1, n_ft):
            for it in proj_items(ftn):
                work.append((2 * ftn, it))

        rows_per_qh = n_st // n_qh
        pass_specs = [
            (qh, ft, parity)
            for qh in range(n_qh)
            for ft in range(n_ft)
            for parity in (0, 1)
        ]
        n_passes = len(pass_specs)
        NEVER = n_passes + 1

        prev = None
        pending_norms = []
        enqueue_next = []  # items appended to `work` at the next pass boundary
        for pidx, (qh, ft, parity) in enumerate(pass_specs):
            for it in enqueue_next:
                work.append((NEVER, it))
            enqueue_next = []
            # force-emit overdue prerequisites (pair projections)
            while work and work[0][0] <= pidx:
                work.popleft()[1]()
            a = AttnQH(ft, qh, parity)
            if pidx == 0:
                stride = 5
            elif qh == 0:
                stride = 3
            else:
                stride = 14
            for kt in range(n_kt):
                if pidx == 0:
                    # finish the upfront while attention runs: remaining
                    # transpose chunks + the v-projections, one per k-tile
                    if n_ch > 2 and kt == 8:
                        upfront_chunk(2, qTp0, kTp0)
                    if n_ch > 3 and kt == 12:
                        upfront_chunk(3, qTp0, kTp0)
                    v_chunk(kt)
                if pending_norms and kt > 0 and kt % 6 == 0:
                    pending_norms.pop(0)()
                if work and kt % stride == stride - 1:
                    work.popleft()[1]()
                a.emit_qk(kt)
                if prev is not None:
                    if prev.emit_pv_one():
                        prev.finish_stage1()
                        pending_norms += prev.normalize_items()
                        if (prev.qh, prev.ft, prev.parity) == (0, n_ft - 1, 1):
                            # first q-half fully normalized soon: release its
                            # out-projection rows into the stream (keep 3 in
                            # reserve to cover the tail's normalize latency)
                            enqueue_next += y_items(range(rows_per_qh - 3))
                        prev = None
                elif len(a.prevs) > AttnQH.LAG:
                    a.emit_pv_one()
            if pidx == 0:
                # x fully transposed: release the row staging, then stage the
                # out-projection weights (first needed half-way through)
                xup_stack.close()
                wo_pool = top.enter_context(tc.tile_pool(name="wo", bufs=1))
                ys_pool_ref.append(
                    top.enter_context(tc.tile_pool(name="ys", bufs=3))
                )
                wo_t.extend(
                    wo_pool.tile([P, DO], dt_o, tag=f"wo{ft2}", name=f"wo{ft2}")
                    for ft2 in range(n_ft)
                )
                for ft2 in range(n_ft):
                    nc.gpsimd.dma_start(
                        wo_t[ft2][:], wout[ft2 * P : (ft2 + 1) * P, :]
                    )
            prev = a

        # tail: flush the last pass, run leftovers, last normalizes, last rows
        while prev.prevs:
            if work:
                work.popleft()[1]()
            prev.emit_pv_one()
        prev.finish_stage1()
        tail_norms = pending_norms + prev.normalize_items(gran=QCH // 2)
        leftovers = [it for _, it in work] + y_items(
            range(rows_per_qh - 3, rows_per_qh), tail=True
        )
        # alternate fine norms with reserved out-projection rows so the PE
        # chews while the DVE reciprocal chain completes
        while tail_norms or leftovers:
            if tail_norms:
                tail_norms.pop(0)()
            if leftovers:
                leftovers.pop(0)()
        for it in y_items(range(rows_per_qh, n_st), tail=True):
            it()

    nc.compile()
    return nc


# problem sizes (hardcoded per contract)
B, S, D, H = 4, 2048, 1024, 16
DO = D
HN = H // 2  # heads per core
SCALE = (D // H) ** -0.5
N_CORES = 8

_NC_CACHE = None


def _get_nc():
    global _NC_CACHE
    if _NC_CACHE is None:
        _NC_CACHE = build_attention(S, D, HN, DO, SCALE)
    return _NC_CACHE


def make_in_maps(x, w_qkv, w_out):
    """Shard full inputs into the 8 per-core input maps (bf16 on host)."""
    import ml_dtypes

    bf16 = ml_dtypes.bfloat16
    xb = x.astype(bf16)
    wb = w_qkv.astype(bf16)
    wob = w_out.astype(bf16)
    in_maps = []
    for c in range(N_CORES):
        b = c // 2
        cs = (c % 2) * HN * HD
        ce = cs + HN * HD
        in_maps.append(
            {
                "x": np.ascontiguousarray(xb[b]),
                "wq": np.ascontiguousarray(wb[:, cs:ce]),
                "wk": np.ascontiguousarray(wb[:, D + cs : D + ce]),
                "wv": np.ascontiguousarray(wb[:, 2 * D + cs : 2 * D + ce]),
                "wout": np.ascontiguousarray(wob[cs:ce, :]),
            }
        )
    return in_maps


def combine_outputs(results, b_out):
    """Sum the two per-batch partials and add the bias."""
    y = np.empty((B, S, DO), dtype=np.float32)
    for b in range(B):
        y[b] = results[2 * b]["y"] + results[2 * b + 1]["y"] + b_out[None, :]
    return y


def kernel(x, w_qkv, w_out, b_out):
    x = np.asarray(x, dtype=np.float32)
    w_qkv = np.asarray(w_qkv, dtype=np.float32)
    w_out = np.asarray(w_out, dtype=np.float32)
    b_out = np.asarray(b_out, dtype=np.float32)
    nc = _get_nc()
    in_maps = make_in_maps(x, w_qkv, w_out)
    res = bass_utils.run_bass_kernel_spmd(nc, in_maps, core_ids=list(range(N_CORES)))
    return combine_outputs(res.results, b_out)
